# revision 1
# baseline (speedup 1.0000x reference)
"""Butterfly sparse-attention MLP kernel for 8 Trainium2 NeuronCores.

Computation (from the reference):
    attn = (w1.T @ w2.T) * sparse_mask          # [4096 s, 4096 t]
    y    = gelu(x @ attn + b2)                  # [8, 768, 4096]

sparse_mask is banded: mask[s, t] == 0 whenever |s - t| > 133.  Each core
owns a 512-wide t-block and only needs an 896-wide s-window around it.
Per t-subtile of 128, only 4 of the 7 s-chunks in the window can carry
non-zero attn, so phase B contracts over 512 of s instead of 4096, and
phase A only computes the in-band t-columns of each attn chunk.

Sharding: tensor-parallel over t (8 blocks of 512).  All per-core variation
is in the input data (windows are zero-padded at the edges; mask zeros make
padded contributions exactly zero), so one SPMD BIR serves all 8 cores.

Matmul operands travel as fp16 (10-bit mantissa; values here are O(1), and
accumulation stays fp32 in PSUM) which halves HBM traffic.  Weight tensors
are host-shuffled so each DMA descriptor is 3.5-4 KB — the HW-DGE queues
are descriptor-rate limited (~60 M/s), not byte limited.  Streams are
spread over the sync/scalar HW-DGE queues plus the gpsimd SW-DGE queue.
"""

import numpy as np

B, T, D = 8, 768, 4096
N = B * T            # 6144 rows of x
NCORES = 8
TB = 512             # t-columns per core
P = 128
MARGIN = 192         # s-window extends this far before/after the t-block
SW = TB + 2 * MARGIN  # 896 s-window width
NCH = SW // P        # 7 s-chunks
DCH = D // P         # 32 d-chunks (contraction of phase A)
NQ = TB // P         # 4 t-subtiles per core
GN = 2048            # n-group width in phase B
NG = N // GN         # 3 n-groups
MMN = 512            # moving-operand / PSUM-bank free-dim cap per matmul
BANDCH = 4           # s-chunks feeding one t-subtile (covers +-133 band)
W1PACK = 2           # w1 d-chunks packed per DMA row (3.5 KB descriptors)
W2PACK = 4           # w2T d-chunks packed per DMA row (4 KB descriptors)

_NC = None


def _band(j):
    """t-column range [lo, hi) of attn chunk j that phase B reads."""
    lo = P * max(0, j - (BANDCH - 1))
    hi = P * min(NQ - 1, j) + P
    return lo, hi


def _build_module():
    from concourse import bacc, bass, mybir, tile
    from concourse.tile_rust import add_dep_helper

    f32 = mybir.dt.float32
    f16 = mybir.dt.float16
    PSUM = bass.MemorySpace.PSUM

    nc = bacc.Bacc("TRN2", target_bir_lowering=False, debug=False)
    xT_d = nc.declare_dram_parameter("xT_s", [NCH, P, N], f16, isOutput=False)
    w1_d = nc.declare_dram_parameter(
        "w1_s", [DCH // W1PACK, P, W1PACK * SW], f16, isOutput=False)
    w2T_d = nc.declare_dram_parameter(
        "w2T_s", [DCH // W2PACK, P, W2PACK * TB], f16, isOutput=False)
    mask_d = nc.declare_dram_parameter("mask_s", [SW, TB], f16, isOutput=False)
    b2_d = nc.declare_dram_parameter("b2c_s", [P, NQ], f32, isOutput=False)
    yT_d = nc.declare_dram_parameter("yT_s", [TB, N], f16, isOutput=True)

    with tile.TileContext(nc) as tc:
        with (
            tc.tile_pool(name="const", bufs=1) as cpool,
            tc.tile_pool(name="attn", bufs=1) as apool,
            tc.tile_pool(name="mp", bufs=1) as mp,
            tc.tile_pool(name="xp", bufs=NG * NCH) as xp,
            tc.tile_pool(name="yp", bufs=6) as yp,
        ):
            b2_t = cpool.tile([P, NQ], f32)
            nc.gpsimd.dma_start(b2_t[:], b2_d[:])

            # Masks land early via the (otherwise idle) SW-DGE queue.
            m_ts = []
            for j in range(NCH):
                m_t = mp.tile([P, TB], f16, name=f"m_t{j}")
                nc.gpsimd.dma_start(m_t[:], mask_d[j * P:(j + 1) * P, :])
                m_ts.append(m_t)

            engs = [nc.sync, nc.scalar, nc.gpsimd]

            # ---- Phase A: attn[s, t] = (w1.T @ w2T) * mask on the band ----
            attn_sb = []
            w1_insts = []
            with (
                tc.tile_pool(name="w1p", bufs=8) as w1p,
                tc.tile_pool(name="w2p", bufs=4) as w2p,
                tc.tile_pool(name="psA", bufs=1, space=PSUM) as psA,
            ):
                attn_ps = [
                    psA.tile([P, TB], f32, name=f"attn_ps{j}") for j in range(NCH)
                ]
                for bb in range(DCH // W2PACK):
                    w2_t = w2p.tile([P, W2PACK * TB], f16)
                    nc.scalar.dma_start(w2_t[:], w2T_d[bb])
                    for hb in range(W2PACK // W1PACK):
                        pi = bb * (W2PACK // W1PACK) + hb
                        w1_t = w1p.tile([P, W1PACK * SW], f16)
                        w1_insts.append(nc.sync.dma_start(w1_t[:], w1_d[pi]))
                        for half in range(W1PACK):
                            k = bb * W2PACK + hb * W1PACK + half
                            w1sl = w1_t[:, half * SW:(half + 1) * SW]
                            w2sl = w2_t[:, (hb * W1PACK + half) * TB:
                                        (hb * W1PACK + half + 1) * TB]
                            for j in (3, 2, 4, 1, 5, 0, 6):
                                lo, hi = _band(j)
                                nc.tensor.matmul(
                                    attn_ps[j][:, lo:hi],
                                    w1sl[:, j * P:(j + 1) * P],
                                    w2sl[:, lo:hi],
                                    start=(k == 0),
                                    stop=(k == DCH - 1),
                                )
                for j in range(NCH):
                    lo, hi = _band(j)
                    a_t = apool.tile([P, TB], f16, name=f"attn_sb{j}")
                    nc.vector.tensor_mul(
                        a_t[:, lo:hi], attn_ps[j][:, lo:hi], m_ts[j][:, lo:hi]
                    )
                    attn_sb.append(a_t)

            # ---- Phase B: yT[t, n] = gelu(attn.T @ xT + b2) on the band ----
            with tc.tile_pool(name="psB", bufs=4, space=PSUM) as psB:
                for g in range(NG):
                    x_t = []
                    gate = {0: 11, 1: 13, 2: 15}[g]
                    for j in range(NCH):
                        xt = xp.tile([P, GN], f16, name="x_t", tag="x_t")
                        xi = nc.gpsimd.dma_start(
                            xt[:], xT_d[j, :, g * GN:(g + 1) * GN]
                        )
                        add_dep_helper(
                            xi.ins, w1_insts[gate].ins,
                            sync=True, reason="pace x prefetch behind w1",
                        )
                        x_t.append(xt)
                    for q in range(NQ):
                        for h in range(GN // (2 * MMN)):
                            y_ps = psB.tile([P, 2 * MMN], f32, name="y_ps",
                                            tag="y_ps")
                            for hh in range(2):
                                osl = slice(hh * MMN, (hh + 1) * MMN)
                                nsl = slice((2 * h + hh) * MMN,
                                            (2 * h + hh + 1) * MMN)
                                for c in range(BANDCH):
                                    j = q + c
                                    nc.tensor.matmul(
                                        y_ps[:, osl],
                                        attn_sb[j][:, q * P:(q + 1) * P],
                                        x_t[j][:, nsl],
                                        start=(c == 0),
                                        stop=(c == BANDCH - 1),
                                    )
                            y_sb = yp.tile([P, 2 * MMN], f16, name="y_sb",
                                           tag="y_sb")
                            nc.scalar.activation(
                                y_sb[:],
                                y_ps[:],
                                mybir.ActivationFunctionType.Gelu,
                                bias=b2_t[:, q:q + 1],
                                scale=1.0,
                            )
                            st_eng = nc.sync if (q + h) % 2 == 0 else nc.scalar
                            st_eng.dma_start(
                                yT_d[q * P:(q + 1) * P,
                                     g * GN + 2 * h * MMN:
                                     g * GN + 2 * (h + 1) * MMN],
                                y_sb[:],
                            )

    nc.compile()
    nc.finalize()
    return nc


def _get_nc():
    global _NC
    if _NC is None:
        _NC = _build_module()
    return _NC


def prepare_in_maps(x, w1, w2, b2, sparse_mask):
    x = np.asarray(x, dtype=np.float32)
    w1 = np.asarray(w1, dtype=np.float32)
    w2 = np.asarray(w2, dtype=np.float32)
    b2 = np.asarray(b2, dtype=np.float32)
    sparse_mask = np.asarray(sparse_mask, dtype=np.float32)

    xT = np.ascontiguousarray(x.reshape(N, D).T.astype(np.float16))   # [s, n]
    w2T = np.ascontiguousarray(w2.T.astype(np.float16))               # [d, t]

    # Zero-pad the s axis by MARGIN on both sides so every core's window is
    # a plain slice; mask zeros make the padded rows contribute nothing.
    xT_pad = np.zeros((D + 2 * MARGIN, N), dtype=np.float16)
    xT_pad[MARGIN:MARGIN + D] = xT
    w1_pad = np.zeros((D, D + 2 * MARGIN), dtype=np.float16)
    w1_pad[:, MARGIN:MARGIN + D] = w1.astype(np.float16)
    mask_pad = np.zeros((D + 2 * MARGIN, D), dtype=np.float16)
    mask_pad[MARGIN:MARGIN + D] = sparse_mask.astype(np.float16)

    in_maps = []
    for i in range(NCORES):
        s0 = i * TB           # window start in padded coords
        t0 = i * TB
        w1win = w1_pad[:, s0:s0 + SW]                     # [D, SW]
        # pack W1PACK d-chunks per DMA row: [DCH/W1PACK, P, W1PACK*SW]
        w1_s = (w1win.reshape(DCH // W1PACK, W1PACK, P, SW)
                .transpose(0, 2, 1, 3)
                .reshape(DCH // W1PACK, P, W1PACK * SW))
        w2win = w2T[:, t0:t0 + TB]                        # [D, TB]
        w2_s = (w2win.reshape(DCH // W2PACK, W2PACK, P, TB)
                .transpose(0, 2, 1, 3)
                .reshape(DCH // W2PACK, P, W2PACK * TB))
        in_maps.append({
            "xT_s": np.ascontiguousarray(
                xT_pad[s0:s0 + SW].reshape(NCH, P, N)),
            "w1_s": np.ascontiguousarray(w1_s),
            "w2T_s": np.ascontiguousarray(w2_s),
            "mask_s": np.ascontiguousarray(mask_pad[s0:s0 + SW, t0:t0 + TB]),
            "b2c_s": np.ascontiguousarray(b2[t0:t0 + TB].reshape(NQ, P).T),
        })
    return in_maps


def assemble(results):
    out = np.empty((N, D), dtype=np.float32)
    for i in range(NCORES):
        out[:, i * TB:(i + 1) * TB] = results[i]["yT_s"].T.astype(np.float32)
    return out.reshape(B, T, D)


def _band_ok(sparse_mask):
    """The Bass kernel only computes attn where each core's 4-chunk window
    covers the mask; verify every mask nonzero falls inside that region."""
    s_idx, t_idx = np.nonzero(np.asarray(sparse_mask) != 0)
    if len(s_idx) == 0:
        return True
    w0 = (t_idx // TB) * TB - MARGIN          # per-core s-window start
    j = (s_idx - w0) // P                     # s-chunk within window
    q = (t_idx % TB) // P                     # t-subtile
    return bool(np.all((j >= q) & (j <= q + BANDCH - 1)
                       & (s_idx >= w0) & (s_idx < w0 + SW)))


def _reference_fallback(x, w1, w2, b2, sparse_mask):
    import jax
    import jax.numpy as jnp

    cpu = jax.devices("cpu")[0]
    with jax.default_device(cpu):
        attn = jnp.einsum("ds,td->st", jnp.asarray(w1), jnp.asarray(w2))
        attn = attn * jnp.asarray(sparse_mask)
        y = jnp.einsum("bds,st->bdt", jnp.asarray(x), attn) + jnp.asarray(b2)
        return np.asarray(jax.nn.gelu(y, approximate=False), dtype=np.float32)


def kernel(x, w1, w2, b2, sparse_mask):
    import time

    from concourse.bass_utils import run_bass_kernel_spmd

    if (np.shape(x) != (B, T, D) or np.shape(w1) != (D, D)
            or np.shape(w2) != (D, D) or np.shape(b2) != (D,)
            or np.shape(sparse_mask) != (D, D) or not _band_ok(sparse_mask)):
        return _reference_fallback(x, w1, w2, b2, sparse_mask)

    in_maps = prepare_in_maps(x, w1, w2, b2, sparse_mask)
    nc = _get_nc()
    last_err = None
    for attempt in range(3):
        try:
            res = run_bass_kernel_spmd(nc, in_maps, list(range(NCORES)))
            return assemble(res.results)
        except Exception as e:  # transient NRT/device errors: retry
            last_err = e
            time.sleep(2.0 * (attempt + 1))
    raise last_err



# revision 4
# speedup vs baseline: 1.2911x; 1.2911x over previous
"""Butterfly sparse-attention MLP kernel for 8 Trainium2 NeuronCores.

Computation (from the reference):
    attn = (w1.T @ w2.T) * sparse_mask          # [4096 s, 4096 t]
    y    = gelu(x @ attn + b2)                  # [8, 768, 4096]

sparse_mask has ~140k nonzeros in 6x6 blocks on five bands (offsets
0/+-64/+-128): attn[s, t] == 0 whenever |s - t| > 133.  Computing attn is
only ~1.2 GFLOP (0.6% of the total work), so the host computes the banded
attn during input prep via batched 6x4096x6 matmuls; the device kernel is
just the 206-GFLOP banded  y = gelu(x @ attn + b2).

Sharding: 4-way tensor-parallel over t (blocks of 1024) x 2-way data
parallel over rows n (blocks of 3072).  Each core loads an 1408-row
s-window of x^T (its 1024 t-block + 133-margin band, padded to 11 chunks
of 128), the banded attn tiles for its block (1 MB), and streams
  yT[q] = gelu(attn[band(q)].T @ x_window + b2)     per 128-t subtile q,
contracting 4 s-chunks of 128 per subtile (band width 394 <= 512).
All operands travel fp16 (accumulation fp32 in PSUM); ~16 MB HBM traffic
and ~102k PE cycles per core, balanced at ~44us each.
"""

import numpy as np

B, T, D = 8, 768, 4096
N = B * T              # 6144 rows of x
NCORES = 8
CT, CN = 4, 2          # t-split x n-split core grid
TB = D // CT           # 1024 t-columns per core
NB = N // CN           # 3072 x-rows per core
P = 128
NQ = TB // P           # 8 t-subtiles per core
MARGIN = 133           # band half-width of attn
NCH = 11               # s-chunks in the x window (128*11 = 1408 >= 1024+2*133)
BANDCH = 4             # s-chunks feeding one t-subtile (394-wide band)
MMN = 512              # moving-operand free-dim cap per matmul
NH = NB // MMN         # 6 n-pieces per core
PADT = MARGIN          # top padding of the s axis
PADROWS = D + P * (NCH - NQ) + PADT  # 4480: fits every core's window

_NC = None


def _build_module():
    from concourse import bacc, bass, mybir, tile

    f32 = mybir.dt.float32
    f16 = mybir.dt.float16
    PSUM = bass.MemorySpace.PSUM

    nc = bacc.Bacc("TRN2", target_bir_lowering=False, debug=False)
    xw_d = nc.declare_dram_parameter("xw_s", [NCH, P, NB], f16, isOutput=False)
    attn_d = nc.declare_dram_parameter(
        "attn_s", [P, NQ * BANDCH * P], f16, isOutput=False)
    b2_d = nc.declare_dram_parameter("b2c_s", [P, NQ], f32, isOutput=False)
    yT_d = nc.declare_dram_parameter("yT_s", [NQ, P, NB], f16, isOutput=True)

    with tile.TileContext(nc) as tc:
        with (
            tc.tile_pool(name="const", bufs=1) as cpool,
            tc.tile_pool(name="attn", bufs=1) as apool,
            tc.tile_pool(name="xp", bufs=1) as xp,
            tc.tile_pool(name="psB", bufs=4, space=PSUM) as psB,
            tc.tile_pool(name="yp", bufs=3) as yp,
        ):
            b2_t = cpool.tile([P, NQ], f32)
            nc.gpsimd.dma_start(b2_t[:], b2_d[:])
            attn_t = apool.tile([P, NQ * BANDCH * P], f16)
            nc.gpsimd.dma_start(attn_t[:], attn_d[:])

            x_t = []
            engs = [nc.sync, nc.scalar, nc.gpsimd]
            for c in range(NCH):
                xt = xp.tile([P, NB], f16, name=f"x_t{c}")
                engs[c % 3].dma_start(xt[:], xw_d[c])
                x_t.append(xt)

            for q in range(NQ):
                pss = [psB.tile([P, 2 * MMN], f32, name="y_ps", tag="y_ps")
                       for _ in range(NH // 2)]
                for c in range(BANDCH):
                    a_sl = attn_t[:, (BANDCH * q + c) * P:
                                  (BANDCH * q + c + 1) * P]
                    for h in range(NH):
                        nc.tensor.matmul(
                            pss[h // 2][:, (h % 2) * MMN:(h % 2 + 1) * MMN],
                            a_sl,
                            x_t[q + c][:, h * MMN:(h + 1) * MMN],
                            start=(c == 0),
                            stop=(c == BANDCH - 1),
                        )
                y_big = yp.tile([P, NB], f16, name="y_sb", tag="y_sb")
                for t3 in range(NH // 2):
                    nc.scalar.activation(
                        y_big[:, t3 * 2 * MMN:(t3 + 1) * 2 * MMN],
                        pss[t3][:],
                        mybir.ActivationFunctionType.Gelu,
                        bias=b2_t[:, q:q + 1],
                        scale=1.0,
                    )
                st_eng = nc.sync if q % 2 == 0 else nc.gpsimd
                st_eng.dma_start(yT_d[q], y_big[:])

    nc.compile()
    nc.finalize()
    return nc


def _get_nc():
    global _NC
    if _NC is None:
        _NC = _build_module()
    return _NC


def _banded_attn(w1, w2, sparse_mask):
    """Host-side: (w1.T @ w2.T) * mask as a padded dense band, fp16.

    Mask nonzeros live in 6x6 blocks at (a + 6j + u, b + 6j + v) for shift
    pairs (a, b) in {(0,0), (64,0), (128,0), (0,64), (0,128)}; each block is
    one 6x4096x6 matmul, batched over j.  Returns [PADROWS, D] fp16 where
    row r corresponds to s = r - PADT.
    """
    PAD = 144  # covers max shift 128 + block overhang 6
    w1p = np.zeros((D, D + PAD), dtype=np.float32)
    w1p[:, :D] = w1
    w2p = np.zeros((D + PAD, D), dtype=np.float32)
    w2p[:D] = w2
    nblk = D // 6 + 1  # 683 blocks of 6 cover every in-range (s, t) block
    attn_pad = np.zeros((PADROWS, D + PAD), dtype=np.float32)
    jj = 6 * np.arange(nblk)
    for a, b in ((0, 0), (64, 0), (128, 0), (0, 64), (0, 128)):
        w1b = np.ascontiguousarray(
            w1p[:, a:a + 6 * nblk].reshape(D, nblk, 6).transpose(1, 2, 0))
        w2b = np.ascontiguousarray(
            w2p[b:b + 6 * nblk].reshape(nblk, 6, D))
        blocks = np.matmul(w1b, w2b.transpose(0, 2, 1))  # [nblk, 6 u, 6 v]
        rows = (PADT + a + jj[:, None] + np.arange(6)[None, :])  # [j, u]
        cols = (b + jj[:, None] + np.arange(6)[None, :])         # [j, v]
        attn_pad[rows[:, :, None], cols[:, None, :]] = blocks
    attn_pad = attn_pad[:, :D]
    attn_pad[:PADT] = 0.0
    attn_pad[PADT + D:] = 0.0
    attn_pad[PADT:PADT + D] *= np.asarray(sparse_mask, dtype=np.float32)
    return attn_pad.astype(np.float16)


def _band_ok(sparse_mask):
    """Every mask nonzero must sit in one of the five 6x6-block bands the
    host attn construction covers (and inside the kernel's s-window)."""
    s_idx, t_idx = np.nonzero(np.asarray(sparse_mask) != 0)
    if len(s_idx) == 0:
        return True
    da = s_idx - 6 * (t_idx // 6)   # type A: t-block-aligned
    db = t_idx - 6 * (s_idx // 6)   # type B: s-block-aligned
    allowed = np.concatenate([np.arange(k, k + 6) for k in (0, 64, 128)])
    return bool(np.all(np.isin(da, allowed) | np.isin(db, allowed)))


def prepare_in_maps(x, w1, w2, b2, sparse_mask):
    x = np.asarray(x, dtype=np.float32)
    w1 = np.asarray(w1, dtype=np.float32)
    w2 = np.asarray(w2, dtype=np.float32)
    b2 = np.asarray(b2, dtype=np.float32)

    attn16 = _banded_attn(w1, w2, sparse_mask)       # [PADROWS, D] fp16

    xT_pad = np.zeros((PADROWS, N), dtype=np.float16)
    xT_pad[PADT:PADT + D] = x.reshape(N, D).T

    in_maps = []
    for i in range(NCORES):
        it, inn = divmod(i, CN)
        t0 = it * TB
        n0 = inn * NB
        xw = np.ascontiguousarray(
            xT_pad[t0:t0 + NCH * P, n0:n0 + NB]).reshape(NCH, P, NB)
        at = np.empty((NQ, BANDCH, P, P), dtype=np.float16)
        for q in range(NQ):
            for c in range(BANDCH):
                r0 = t0 + P * (q + c)
                at[q, c] = attn16[r0:r0 + P, t0 + P * q:t0 + P * (q + 1)]
        attn_s = np.ascontiguousarray(
            at.transpose(2, 0, 1, 3).reshape(P, NQ * BANDCH * P))
        in_maps.append({
            "xw_s": xw,
            "attn_s": attn_s,
            "b2c_s": np.ascontiguousarray(b2[t0:t0 + TB].reshape(NQ, P).T),
        })
    return in_maps


def assemble(results):
    out = np.empty((N, D), dtype=np.float32)
    for i in range(NCORES):
        it, inn = divmod(i, CN)
        t0 = it * TB
        n0 = inn * NB
        yT = results[i]["yT_s"]                      # [NQ, P, NB]
        out[n0:n0 + NB, t0:t0 + TB] = (
            yT.transpose(2, 0, 1).reshape(NB, TB).astype(np.float32))
    return out.reshape(B, T, D)


def _reference_fallback(x, w1, w2, b2, sparse_mask):
    import jax
    import jax.numpy as jnp

    cpu = jax.devices("cpu")[0]
    with jax.default_device(cpu):
        attn = jnp.einsum("ds,td->st", jnp.asarray(w1), jnp.asarray(w2))
        attn = attn * jnp.asarray(sparse_mask)
        y = jnp.einsum("bds,st->bdt", jnp.asarray(x), attn) + jnp.asarray(b2)
        return np.asarray(jax.nn.gelu(y, approximate=False), dtype=np.float32)


def kernel(x, w1, w2, b2, sparse_mask):
    import time

    from concourse.bass_utils import run_bass_kernel_spmd

    if (np.shape(x) != (B, T, D) or np.shape(w1) != (D, D)
            or np.shape(w2) != (D, D) or np.shape(b2) != (D,)
            or np.shape(sparse_mask) != (D, D) or not _band_ok(sparse_mask)):
        return _reference_fallback(x, w1, w2, b2, sparse_mask)

    in_maps = prepare_in_maps(x, w1, w2, b2, sparse_mask)
    nc = _get_nc()
    last_err = None
    for attempt in range(3):
        try:
            res = run_bass_kernel_spmd(nc, in_maps, list(range(NCORES)))
            return assemble(res.results)
        except Exception as e:  # transient NRT/device errors: retry
            last_err = e
            time.sleep(2.0 * (attempt + 1))
    raise last_err
